# revision 2
# baseline (speedup 1.0000x reference)
# MoE kernel for Trainium2 (8 NeuronCores, dff-sharded / intra-expert tensor
# parallel).
#
# Strategy:
#  - Host: gate logits = x @ gate_w, top-2 + softmax, gather tokens per expert.
#  - Each core gets a 512-wide dff slice of EVERY expert (w1[:, c*512:(c+1)*512],
#    w2[c*512:(c+1)*512, :]) and processes ALL routed tokens on that slice.
#    Every core therefore does identical work: sum_e L_e = 8192 token-slots x
#    64 PE-cycles/slot = 218.5 us of bf16 matmul -- zero load-imbalance waste
#    (vs. expert-per-core, which pays max_e L_e x 512 cycles).
#  - Device per expert-slot, per token-tile g (<=512 tokens):
#    GEMM1 h = gelu(x^T-major @ w1-slice + b1-slice); GEMM2 y_partial = h @ w2-
#    slice.  GEMM1(g) and GEMM2(g-1) are software-pipelined so ACT gelu latency
#    never stalls the PE.  PE is prewarmed with dummy matmuls during the input
#    DMA so HAM is at full clock when real work starts.
#  - Host: sum the 8 partial y's (disjoint dff slices of the same tokens),
#    add b2, scale by gate weights, scatter-add into the output.
import math
from contextlib import ExitStack

import ml_dtypes
import numpy as np

import concourse.bass as bass
import concourse.mybir as mybir
import concourse.tile as tile
from concourse.bass_utils import run_bass_kernel_spmd

D = 1024
DFF = 4096
E = 8
TOP_K = 2
P = 128
KD = D // P        # 8 contraction tiles for GEMM1
S = DFF // 8       # 512 dff columns per core
NF_S = S // P      # 4 f-tiles per expert slot
ND = D // P        # 8 GEMM2 out tiles
T_TILE = 512
WARM_MM = 14       # dummy matmuls to warm the PE/HAM during input DMA

BF16 = mybir.dt.bfloat16
F32 = mybir.dt.float32
NP_BF16 = np.dtype(ml_dtypes.bfloat16)

_neff_cache = {}


def _t_sizes(L):
    """Split L tokens into ceil(L/512) near-equal tiles (sizes sum to L)."""
    n = max(1, math.ceil(L / T_TILE))
    base, rem = divmod(L, n)
    return [base + 1] * rem + [base] * (n - rem)


def _split_multiwait_json(bir_bytes: bytes) -> bytes:
    """The walrus build in this container rejects instructions carrying more
    than one sync wait (or update). Split extras onto adjacent single-wait
    EventSemaphore carriers on the same engine: program order on the engine
    preserves the semantics exactly."""
    import json as _json

    bir = _json.loads(bir_bytes)
    for fn in bir["functions"]:
        for blk in fn["blocks"]:
            insts = blk.get("instructions", [])
            out = []
            for inst in insts:
                si = inst.get("sync_info")
                if si:
                    waits = si.get("on_wait") or []
                    if len(waits) > 1:
                        for i, w in enumerate(waits[:-1]):
                            out.append({
                                "debug": inst.get("debug", 0),
                                "engine": inst["engine"],
                                "ins": [],
                                "name": f"{inst['name']}_w{i}",
                                "opcode": "EventSemaphore",
                                "outs": [],
                                "sync_info": {"on_update": [], "on_wait": [w]},
                            })
                        si["on_wait"] = [waits[-1]]
                out.append(inst)
                if si:
                    ups = si.get("on_update") or []
                    if len(ups) > 1:
                        for i, u in enumerate(ups[1:]):
                            out.append({
                                "debug": inst.get("debug", 0),
                                "engine": inst["engine"],
                                "ins": [],
                                "name": f"{inst['name']}_u{i}",
                                "opcode": "EventSemaphore",
                                "outs": [],
                                "sync_info": {"on_update": [u], "on_wait": []},
                            })
                        si["on_update"] = [ups[0]]
            blk["instructions"] = out
    return _json.dumps(bir).encode()


def _patch_to_json(nc: bass.Bass) -> bass.Bass:
    orig = nc.to_json_bytes
    nc.to_json_bytes = lambda: _split_multiwait_json(orig())
    return nc


def _build_bass(slot_tiles):
    """slot_tiles: list (one entry per active expert slot) of tile-size lists.

    DRAM layouts (host pre-blocks so every DMA reads large contiguous runs):
      xs : [G*P, KD*512] bf16; row g*P+p, cols 0:KD*tsz = x[tok, k*P+p] packed
           (kd-major, token minor) -- one contiguous KD*tsz run per partition.
      w1 : [EA*P, NF_S*KD*P] bf16; row s*P+p, col (f,k,m) = w1slice[k*P+p, f*P+m]
      w2 : [EA*P, NF_S*ND*P] bf16; row s*P+p, col (f,dd,m) = w2slice[f*P+p, dd*P+m]
      b1 : [EA*NF_S*P] f32, (slot,f,p) order
    Output:
      y  : [G*P, ND*512] bf16; row g*P+p, col dd*tsz+c = y_partial[tok c, dd*P+p]
    """
    nc = bass.Bass()
    EA = len(slot_tiles)
    gl = []  # (slot, first_of_slot, tsz)
    for s, sizes in enumerate(slot_tiles):
        for i, tsz in enumerate(sizes):
            gl.append((s, i == 0, tsz))
    G = len(gl)

    xs_h = nc.dram_tensor("xs", [G * P, KD * T_TILE], BF16, kind="ExternalInput")
    w1_h = nc.dram_tensor("w1", [EA * P, NF_S * KD * P], BF16, kind="ExternalInput")
    w2_h = nc.dram_tensor("w2", [EA * P, NF_S * ND * P], BF16, kind="ExternalInput")
    b1_h = nc.dram_tensor("b1", [EA * NF_S * P], F32, kind="ExternalInput")
    y_h = nc.dram_tensor("y", [G * P, ND * T_TILE], BF16, kind="ExternalOutput")
    warm_h = nc.dram_tensor("warm", [P, 4], F32, kind="ExternalOutput")

    gelu = mybir.ActivationFunctionType.Gelu

    with ExitStack() as ctx:
        tc = ctx.enter_context(tile.TileContext(nc))
        zpool = ctx.enter_context(tc.tile_pool(name="z", bufs=1))
        wpool = ctx.enter_context(tc.tile_pool(name="w", bufs=3))
        xpool = ctx.enter_context(tc.tile_pool(name="x", bufs=4))
        hpool = ctx.enter_context(tc.tile_pool(name="h", bufs=2))
        bpool = ctx.enter_context(tc.tile_pool(name="b", bufs=1))
        ypool = ctx.enter_context(tc.tile_pool(name="y", bufs=2))
        pwarm = ctx.enter_context(tc.tile_pool(name="pw", bufs=1, space="PSUM"))
        ps1 = ctx.enter_context(tc.tile_pool(name="ps1", bufs=2, space="PSUM"))
        ps2 = ctx.enter_context(tc.tile_pool(name="ps2", bufs=2, space="PSUM"))

        # --- PE prewarm: dummy matmuls on zeroed tiles keep the PE busy from
        # ~t=6.5us so the HAM clock gate is at 8/8 when real matmuls start.
        zw = zpool.tile([P, P], BF16, name="zw")
        zx = zpool.tile([P, T_TILE], BF16, name="zx")
        nc.vector.memset(zw[:], 0)
        nc.vector.memset(zx[:], 0)
        pw = pwarm.tile([P, T_TILE], F32, name="pw")
        for i in range(WARM_MM):
            nc.tensor.matmul(pw[:], zw[:], zx[:],
                             start=(i == 0), stop=(i == WARM_MM - 1))
        ws = zpool.tile([P, 4], F32, name="ws")
        nc.vector.tensor_copy(ws[:], pw[:, :4])
        nc.sync.dma_start(warm_h[:, :], ws[:])

        # --- critical-path DMAs first (emission order == scheduler priority):
        # x tile 0 and slot-0 weights gate the first real matmul.
        def dma_x(g, tsz):
            t = xpool.tile([P, KD, T_TILE], BF16, tag="x", name=f"x{g}")
            nc.sync.dma_start(
                t[:, :, :tsz],
                xs_h[g * P:(g + 1) * P, :KD * tsz].rearrange(
                    "p (kd c) -> p kd c", kd=KD),
            )
            return t

        def dma_w(s):
            t1 = wpool.tile([P, NF_S * KD * P], BF16, tag="w1", name=f"w1_{s}")
            nc.scalar.dma_start(t1[:], w1_h[s * P:(s + 1) * P, :])
            t2 = wpool.tile([P, NF_S * ND * P], BF16, tag="w2", name=f"w2_{s}")
            nc.scalar.dma_start(t2[:], w2_h[s * P:(s + 1) * P, :])
            return t1, t2

        x_t = [None] * G
        x_t[0] = dma_x(0, gl[0][2])
        w_t = [None] * EA
        w_t[0] = dma_w(0)
        b1_raw = bpool.tile([P, EA * NF_S], F32, name="b1r")
        nc.gpsimd.dma_start(b1_raw[:], b1_h[:].rearrange("(g p) -> p g", p=P))
        # Funnel b1 through an ACT-engine copy: downstream gelus then reach it
        # via same-engine program order instead of an extra semaphore wait.
        b1_t = bpool.tile([P, EA * NF_S], F32, name="b1c")
        nc.scalar.copy(b1_t[:], b1_raw[:])
        if G > 1:
            x_t[1] = dma_x(1, gl[1][2])

        # --- main loop: GEMM1(g) then GEMM2(g-1), pipelined so the gelu of
        # tile g's last f-block completes while GEMM2(g-1) occupies the PE.
        h_t = [None] * G

        def gemm2(j):
            s, _, tsz = gl[j]
            yst = ypool.tile([P, ND, T_TILE], BF16, tag="yst", name=f"y{j}")
            for dd in range(ND):
                pt2 = ps2.tile([P, T_TILE], F32, tag="ps2", name="pt2")
                for f in range(NF_S):
                    nc.tensor.matmul(
                        pt2[:, :tsz],
                        w_t[s][1][:, (f * ND + dd) * P:(f * ND + dd + 1) * P],
                        h_t[j][f][:, :tsz],
                        start=(f == 0),
                        stop=(f == NF_S - 1),
                    )
                nc.vector.tensor_copy(yst[:, dd, :tsz], pt2[:, :tsz])
            nc.sync.dma_start(
                y_h[j * P:(j + 1) * P, :ND * tsz].rearrange(
                    "p (dd c) -> p dd c", dd=ND),
                yst[:, :, :tsz],
            )
            h_t[j] = None

        for g, (s, first, tsz) in enumerate(gl):
            if first and s + 1 < EA:
                w_t[s + 1] = dma_w(s + 1)
            if g + 2 < G:
                x_t[g + 2] = dma_x(g + 2, gl[g + 2][2])
            hs = [hpool.tile([P, T_TILE], BF16, tag=f"h{f}", name=f"h{g}_{f}")
                  for f in range(NF_S)]
            for f in range(NF_S):
                pt = ps1.tile([P, T_TILE], F32, tag="ps1", name="pt1")
                for k in range(KD):
                    nc.tensor.matmul(
                        pt[:, :tsz],
                        w_t[s][0][:, (f * KD + k) * P:(f * KD + k + 1) * P],
                        x_t[g][:, k, :tsz],
                        start=(k == 0),
                        stop=(k == KD - 1),
                    )
                nc.scalar.activation(
                    hs[f][:, :tsz], pt[:, :tsz], gelu,
                    bias=b1_t[:, s * NF_S + f:s * NF_S + f + 1],
                )
            h_t[g] = hs
            x_t[g] = None
            if g > 0:
                gemm2(g - 1)
        gemm2(G - 1)
    return _patch_to_json(nc)


def _route(xf: np.ndarray, gate_w: np.ndarray):
    """Top-2 gating identical to the reference (argmax ties -> lower index)."""
    N = xf.shape[0]
    logits = xf @ gate_w  # (N, E) f32
    rows = np.arange(N)
    i1 = logits.argmax(1)
    v1 = logits[rows, i1]
    masked = logits.copy()
    masked[rows, i1] = -np.inf
    i2 = masked.argmax(1)
    v2 = masked[rows, i2]
    # softmax over the two selected logits (v1 >= v2)
    e = np.exp((v2 - v1).astype(np.float32))
    wt1 = (1.0 / (1.0 + e)).astype(np.float32)
    wt2 = (e / (1.0 + e)).astype(np.float32)
    idx_e, wts_e = [], []
    for ex in range(E):
        s1 = np.nonzero(i1 == ex)[0]
        s2 = np.nonzero(i2 == ex)[0]
        idx_e.append(np.concatenate([s1, s2]))
        wts_e.append(np.concatenate([wt1[s1], wt2[s2]]).astype(np.float32))
    return idx_e, wts_e


def kernel(x, gate_w, w1, b1, w2, b2, _trace=False):
    B, T, D_ = x.shape
    N = B * T
    xf = np.ascontiguousarray(x.reshape(N, D_).astype(np.float32))
    idx_e, wts_e = _route(xf, gate_w.astype(np.float32))
    cnts = np.array([len(i) for i in idx_e])
    order = np.argsort(-cnts, kind="stable")
    order = [int(e) for e in order if cnts[e] > 0]
    slot_tiles = [_t_sizes(int(cnts[e])) for e in order]
    EA = len(order)
    gl = []  # (slot, t0_within_expert, tsz)
    for s, sizes in enumerate(slot_tiles):
        t0 = 0
        for tsz in sizes:
            gl.append((s, t0, tsz))
            t0 += tsz
    G = len(gl)

    key = tuple(tuple(st) for st in slot_tiles)
    if key in _neff_cache:
        nc = _neff_cache[key]
    else:
        nc = _build_bass(slot_tiles)
        _neff_cache[key] = nc

    # --- host-side pre-blocking (shared across cores for xs, per-core for w)
    xs = np.zeros((G * P, KD * T_TILE), NP_BF16)
    xg_by_slot = [xf[idx_e[e]] for e in order]
    for g, (s, t0, tsz) in enumerate(gl):
        blk = xg_by_slot[s][t0:t0 + tsz]                    # [tsz, D] f32
        b3 = blk.T.reshape(KD, P, tsz).transpose(1, 0, 2)   # [P, KD, tsz]
        xs[g * P:(g + 1) * P, :KD * tsz] = (
            b3.reshape(P, KD * tsz).astype(NP_BF16))

    in_maps = []
    for c in range(8):
        cS = c * S
        w1s = np.empty((EA * P, NF_S * KD * P), NP_BF16)
        w2s = np.empty((EA * P, NF_S * ND * P), NP_BF16)
        b1s = np.empty(EA * NF_S * P, np.float32)
        for s, e in enumerate(order):
            a = w1[e][:, cS:cS + S]                          # [D, S]
            w1s[s * P:(s + 1) * P] = (
                a.reshape(KD, P, NF_S, P).transpose(1, 2, 0, 3)
                .reshape(P, NF_S * KD * P).astype(NP_BF16))
            bslc = w2[e][cS:cS + S, :]                       # [S, D]
            w2s[s * P:(s + 1) * P] = (
                bslc.reshape(NF_S, P, ND, P).transpose(1, 0, 2, 3)
                .reshape(P, NF_S * ND * P).astype(NP_BF16))
            b1s[s * NF_S * P:(s + 1) * NF_S * P] = b1[e][cS:cS + S]
        in_maps.append({
            "xs": xs,
            "w1": w1s,
            "w2": w2s,
            "b1": np.ascontiguousarray(b1s),
        })

    res = run_bass_kernel_spmd(nc, in_maps, core_ids=list(range(8)),
                               trace=_trace)
    if _trace:
        print(f"HW exec time: {res.exec_time_ns} ns")

    # --- unshard: sum the 8 dff-slice partials, then combine + scatter-add
    ysum = np.zeros((G * P, ND * T_TILE), np.float32)
    for c in range(8):
        ysum += res.results[c]["y"].astype(np.float32)

    out = np.zeros((N, D), np.float32)
    for s, e in enumerate(order):
        L = int(cnts[e])
        ye = np.empty((L, D), np.float32)
        for g, (sg, t0, tsz) in enumerate(gl):
            if sg != s:
                continue
            blk = ysum[g * P:(g + 1) * P, :ND * tsz]
            ye[t0:t0 + tsz] = (
                blk.reshape(P, ND, tsz).transpose(2, 1, 0).reshape(tsz, D))
        yv = ye + b2[e][None, :].astype(np.float32)
        out[idx_e[e]] += wts_e[e][:, None] * yv
    return out.reshape(B, T, D_)


# revision 5
# speedup vs baseline: 1.0364x; 1.0364x over previous
# MoE kernel for Trainium2 (8 NeuronCores, dff-sharded / intra-expert tensor
# parallel).
#
# Strategy:
#  - Host: gate logits = x @ gate_w, top-2 + softmax, gather tokens per expert.
#  - Each core gets a 512-wide dff slice of EVERY expert (w1[:, c*512:(c+1)*512],
#    w2[c*512:(c+1)*512, :]) and processes ALL routed tokens on that slice.
#    Every core therefore does identical work: sum_e L_e = 8192 token-slots x
#    64 PE-cycles/slot = 218.5 us of bf16 matmul -- zero load-imbalance waste
#    (vs. expert-per-core, which pays max_e L_e x 512 cycles).
#  - Device per expert-slot, per token-tile g (<=512 tokens):
#    GEMM1 h = gelu(x^T-major @ w1-slice + b1-slice); GEMM2 y_partial = h @ w2-
#    slice.  GEMM1(g) and GEMM2(g-1) are software-pipelined so ACT gelu latency
#    never stalls the PE.  PE is prewarmed with dummy matmuls during the input
#    DMA so HAM is at full clock when real work starts.
#  - Host: sum the 8 partial y's (disjoint dff slices of the same tokens),
#    add b2, scale by gate weights, scatter-add into the output.
import math
from contextlib import ExitStack

import ml_dtypes
import numpy as np

import concourse.bass as bass
import concourse.mybir as mybir
import concourse.tile as tile
from concourse.bass_utils import run_bass_kernel_spmd

D = 1024
DFF = 4096
E = 8
TOP_K = 2
P = 128
KD = D // P        # 8 contraction tiles for GEMM1
S = DFF // 8       # 512 dff columns per core
NF_S = S // P      # 4 f-tiles per expert slot
ND = D // P        # 8 GEMM2 out tiles
T_TILE = 512
WARM_MM = 14       # dummy matmuls to warm the PE/HAM during input DMA

BF16 = mybir.dt.bfloat16
F32 = mybir.dt.float32
NP_BF16 = np.dtype(ml_dtypes.bfloat16)

_neff_cache = {}


def _t_sizes(L):
    """Split L tokens into ceil(L/512) near-equal tiles (sizes sum to L)."""
    n = max(1, math.ceil(L / T_TILE))
    base, rem = divmod(L, n)
    return [base + 1] * rem + [base] * (n - rem)


def _split_multiwait_json(bir_bytes: bytes) -> bytes:
    """The walrus build in this container rejects instructions carrying more
    than one sync wait (or update). Split extras onto adjacent single-wait
    EventSemaphore carriers on the same engine: program order on the engine
    preserves the semantics exactly."""
    import json as _json

    bir = _json.loads(bir_bytes)
    for fn in bir["functions"]:
        for blk in fn["blocks"]:
            insts = blk.get("instructions", [])
            out = []
            for inst in insts:
                si = inst.get("sync_info")
                if si:
                    waits = si.get("on_wait") or []
                    if len(waits) > 1:
                        for i, w in enumerate(waits[:-1]):
                            out.append({
                                "debug": inst.get("debug", 0),
                                "engine": inst["engine"],
                                "ins": [],
                                "name": f"{inst['name']}_w{i}",
                                "opcode": "EventSemaphore",
                                "outs": [],
                                "sync_info": {"on_update": [], "on_wait": [w]},
                            })
                        si["on_wait"] = [waits[-1]]
                out.append(inst)
                if si:
                    ups = si.get("on_update") or []
                    if len(ups) > 1:
                        for i, u in enumerate(ups[1:]):
                            out.append({
                                "debug": inst.get("debug", 0),
                                "engine": inst["engine"],
                                "ins": [],
                                "name": f"{inst['name']}_u{i}",
                                "opcode": "EventSemaphore",
                                "outs": [],
                                "sync_info": {"on_update": [u], "on_wait": []},
                            })
                        si["on_update"] = [ups[0]]
            blk["instructions"] = out
    return _json.dumps(bir).encode()


def _patch_to_json(nc: bass.Bass) -> bass.Bass:
    orig = nc.to_json_bytes
    nc.to_json_bytes = lambda: _split_multiwait_json(orig())
    return nc


def _build_bass(slot_tiles):
    """slot_tiles: list (one entry per active expert slot) of tile-size lists.

    DRAM layouts (host pre-blocks so every DMA reads large contiguous runs):
      xs : [G*P, KD*512] bf16; row g*P+p, cols 0:KD*tsz = x[tok, k*P+p] packed
           (kd-major, token minor) -- one contiguous KD*tsz run per partition.
      w1 : [EA*P, NF_S*KD*P] bf16; row s*P+p, col (f,k,m) = w1slice[k*P+p, f*P+m]
      w2 : [EA*P, NF_S*ND*P] bf16; row s*P+p, col (f,dd,m) = w2slice[f*P+p, dd*P+m]
      b1 : [EA*NF_S*P] f32, (slot,f,p) order
    Output:
      y  : [G*P, ND*512] bf16; row g*P+p, col dd*tsz+c = y_partial[tok c, dd*P+p]
    """
    nc = bass.Bass()
    EA = len(slot_tiles)
    gl = []  # (slot, first_of_slot, tsz)
    for s, sizes in enumerate(slot_tiles):
        for i, tsz in enumerate(sizes):
            gl.append((s, i == 0, tsz))
    G = len(gl)

    xs_h = nc.dram_tensor("xs", [G * P, KD * T_TILE], BF16, kind="ExternalInput")
    w1_h = nc.dram_tensor("w1", [EA * P, NF_S * KD * P], BF16, kind="ExternalInput")
    w2_h = nc.dram_tensor("w2", [EA * P, NF_S * ND * P], BF16, kind="ExternalInput")
    b1_h = nc.dram_tensor("b1", [EA * NF_S * P], F32, kind="ExternalInput")
    y_h = nc.dram_tensor("y", [G * P, ND * T_TILE], BF16, kind="ExternalOutput")
    warm_h = nc.dram_tensor("warm", [P, 4], F32, kind="ExternalOutput")

    gelu = mybir.ActivationFunctionType.Gelu

    with ExitStack() as ctx:
        tc = ctx.enter_context(tile.TileContext(nc))
        zpool = ctx.enter_context(tc.tile_pool(name="z", bufs=1))
        wpool = ctx.enter_context(tc.tile_pool(name="w", bufs=3))
        xpool = ctx.enter_context(tc.tile_pool(name="x", bufs=4))
        hpool = ctx.enter_context(tc.tile_pool(name="h", bufs=2))
        bpool = ctx.enter_context(tc.tile_pool(name="b", bufs=1))
        ypool = ctx.enter_context(tc.tile_pool(name="y", bufs=2))
        pwarm = ctx.enter_context(tc.tile_pool(name="pw", bufs=1, space="PSUM"))
        ps1 = ctx.enter_context(tc.tile_pool(name="ps1", bufs=2, space="PSUM"))
        ps2 = ctx.enter_context(tc.tile_pool(name="ps2", bufs=2, space="PSUM"))

        # --- PE prewarm: dummy matmuls on zeroed tiles keep the PE busy from
        # ~t=6.5us so the HAM clock gate is at 8/8 when real matmuls start.
        zw = zpool.tile([P, P], BF16, name="zw")
        zx = zpool.tile([P, T_TILE], BF16, name="zx")
        nc.vector.memset(zw[:], 0)
        nc.vector.memset(zx[:], 0)
        pw = pwarm.tile([P, T_TILE], F32, name="pw")
        for i in range(WARM_MM):
            nc.tensor.matmul(pw[:], zw[:], zx[:],
                             start=(i == 0), stop=(i == WARM_MM - 1))
        ws = zpool.tile([P, 4], F32, name="ws")
        nc.vector.tensor_copy(ws[:], pw[:, :4])
        nc.sync.dma_start(warm_h[:, :], ws[:])

        # --- critical-path DMAs first (emission order == scheduler priority):
        # x tile 0 and slot-0 weights gate the first real matmul.
        def dma_x(g, tsz):
            # Contiguous tsz-packed layout on both sides: one 5.8KB run per
            # partition per DMA descriptor (vs 8x 728B runs for a 3D slice).
            t = xpool.tile([P, KD * T_TILE], BF16, tag="x", name=f"x{g}")
            nc.sync.dma_start(
                t[:, :KD * tsz], xs_h[g * P:(g + 1) * P, :KD * tsz])
            return t

        def dma_w(s):
            t1 = wpool.tile([P, NF_S * KD * P], BF16, tag="w1", name=f"w1_{s}")
            nc.scalar.dma_start(t1[:], w1_h[s * P:(s + 1) * P, :])
            t2 = wpool.tile([P, NF_S * ND * P], BF16, tag="w2", name=f"w2_{s}")
            nc.scalar.dma_start(t2[:], w2_h[s * P:(s + 1) * P, :])
            return t1, t2

        x_t = [None] * G
        x_t[0] = dma_x(0, gl[0][2])
        w_t = [None] * EA
        w_t[0] = dma_w(0)
        b1_raw = bpool.tile([P, EA * NF_S], F32, name="b1r")
        nc.gpsimd.dma_start(b1_raw[:], b1_h[:].rearrange("(g p) -> p g", p=P))
        # Funnel b1 through an ACT-engine copy: downstream gelus then reach it
        # via same-engine program order instead of an extra semaphore wait.
        b1_t = bpool.tile([P, EA * NF_S], F32, name="b1c")
        nc.scalar.copy(b1_t[:], b1_raw[:])
        if G > 1:
            x_t[1] = dma_x(1, gl[1][2])

        # --- main loop: GEMM1(g) then GEMM2(g-1), pipelined so the gelu of
        # tile g's last f-block completes while GEMM2(g-1) occupies the PE.
        h_t = [None] * G

        def gemm2(j):
            s, _, tsz = gl[j]
            last = j == G - 1
            yst = ypool.tile([P, ND * T_TILE], BF16, tag="yst", name=f"y{j}")
            for dd in range(ND):
                pt2 = ps2.tile([P, T_TILE], F32, tag="ps2", name="pt2")
                for f in range(NF_S):
                    nc.tensor.matmul(
                        pt2[:, :tsz],
                        w_t[s][1][:, (f * ND + dd) * P:(f * ND + dd + 1) * P],
                        h_t[j][f][:, :tsz],
                        start=(f == 0),
                        stop=(f == NF_S - 1),
                    )
                nc.vector.tensor_copy(
                    yst[:, dd * tsz:(dd + 1) * tsz], pt2[:, :tsz])
                if last:
                    # Per-dd stores on the final tile: the kernel tail only
                    # waits on one 128-col-slab DMA instead of the full tile.
                    nc.sync.dma_start(
                        y_h[j * P:(j + 1) * P, dd * tsz:(dd + 1) * tsz],
                        yst[:, dd * tsz:(dd + 1) * tsz])
            if not last:
                nc.sync.dma_start(
                    y_h[j * P:(j + 1) * P, :ND * tsz], yst[:, :ND * tsz])
            h_t[j] = None

        for g, (s, first, tsz) in enumerate(gl):
            if first and s + 1 < EA:
                w_t[s + 1] = dma_w(s + 1)
            if g + 2 < G:
                x_t[g + 2] = dma_x(g + 2, gl[g + 2][2])
            hs = [hpool.tile([P, T_TILE], BF16, tag=f"h{f}", name=f"h{g}_{f}")
                  for f in range(NF_S)]
            for f in range(NF_S):
                pt = ps1.tile([P, T_TILE], F32, tag="ps1", name="pt1")
                for k in range(KD):
                    nc.tensor.matmul(
                        pt[:, :tsz],
                        w_t[s][0][:, (f * KD + k) * P:(f * KD + k + 1) * P],
                        x_t[g][:, k * tsz:(k + 1) * tsz],
                        start=(k == 0),
                        stop=(k == KD - 1),
                    )
                nc.scalar.activation(
                    hs[f][:, :tsz], pt[:, :tsz], gelu,
                    bias=b1_t[:, s * NF_S + f:s * NF_S + f + 1],
                )
            h_t[g] = hs
            x_t[g] = None
            if g > 0:
                gemm2(g - 1)
        gemm2(G - 1)
    return _patch_to_json(nc)


def _route(xf: np.ndarray, gate_w: np.ndarray):
    """Top-2 gating identical to the reference (argmax ties -> lower index)."""
    N = xf.shape[0]
    logits = xf @ gate_w  # (N, E) f32
    rows = np.arange(N)
    i1 = logits.argmax(1)
    v1 = logits[rows, i1]
    masked = logits.copy()
    masked[rows, i1] = -np.inf
    i2 = masked.argmax(1)
    v2 = masked[rows, i2]
    # softmax over the two selected logits (v1 >= v2)
    e = np.exp((v2 - v1).astype(np.float32))
    wt1 = (1.0 / (1.0 + e)).astype(np.float32)
    wt2 = (e / (1.0 + e)).astype(np.float32)
    idx_e, wts_e = [], []
    for ex in range(E):
        s1 = np.nonzero(i1 == ex)[0]
        s2 = np.nonzero(i2 == ex)[0]
        idx_e.append(np.concatenate([s1, s2]))
        wts_e.append(np.concatenate([wt1[s1], wt2[s2]]).astype(np.float32))
    return idx_e, wts_e


def kernel(x, gate_w, w1, b1, w2, b2, _trace=False):
    B, T, D_ = x.shape
    N = B * T
    xf = np.ascontiguousarray(x.reshape(N, D_).astype(np.float32))
    idx_e, wts_e = _route(xf, gate_w.astype(np.float32))
    cnts = np.array([len(i) for i in idx_e])
    order = np.argsort(-cnts, kind="stable")
    order = [int(e) for e in order if cnts[e] > 0]
    slot_tiles = [_t_sizes(int(cnts[e])) for e in order]
    EA = len(order)
    gl = []  # (slot, t0_within_expert, tsz)
    for s, sizes in enumerate(slot_tiles):
        t0 = 0
        for tsz in sizes:
            gl.append((s, t0, tsz))
            t0 += tsz
    G = len(gl)

    key = tuple(tuple(st) for st in slot_tiles)
    if key in _neff_cache:
        nc = _neff_cache[key]
    else:
        nc = _build_bass(slot_tiles)
        _neff_cache[key] = nc

    # --- host-side pre-blocking (shared across cores for xs, per-core for w)
    xs = np.zeros((G * P, KD * T_TILE), NP_BF16)
    xg_by_slot = [xf[idx_e[e]] for e in order]
    for g, (s, t0, tsz) in enumerate(gl):
        blk = xg_by_slot[s][t0:t0 + tsz]                    # [tsz, D] f32
        b3 = blk.T.reshape(KD, P, tsz).transpose(1, 0, 2)   # [P, KD, tsz]
        xs[g * P:(g + 1) * P, :KD * tsz] = (
            b3.reshape(P, KD * tsz).astype(NP_BF16))

    in_maps = []
    for c in range(8):
        cS = c * S
        w1s = np.empty((EA * P, NF_S * KD * P), NP_BF16)
        w2s = np.empty((EA * P, NF_S * ND * P), NP_BF16)
        b1s = np.empty(EA * NF_S * P, np.float32)
        for s, e in enumerate(order):
            a = w1[e][:, cS:cS + S]                          # [D, S]
            w1s[s * P:(s + 1) * P] = (
                a.reshape(KD, P, NF_S, P).transpose(1, 2, 0, 3)
                .reshape(P, NF_S * KD * P).astype(NP_BF16))
            bslc = w2[e][cS:cS + S, :]                       # [S, D]
            w2s[s * P:(s + 1) * P] = (
                bslc.reshape(NF_S, P, ND, P).transpose(1, 0, 2, 3)
                .reshape(P, NF_S * ND * P).astype(NP_BF16))
            b1s[s * NF_S * P:(s + 1) * NF_S * P] = b1[e][cS:cS + S]
        in_maps.append({
            "xs": xs,
            "w1": w1s,
            "w2": w2s,
            "b1": np.ascontiguousarray(b1s),
        })

    res = run_bass_kernel_spmd(nc, in_maps, core_ids=list(range(8)),
                               trace=_trace)
    if _trace:
        print(f"HW exec time: {res.exec_time_ns} ns")

    # --- unshard: sum the 8 dff-slice partials, then combine + scatter-add
    ysum = np.zeros((G * P, ND * T_TILE), np.float32)
    for c in range(8):
        ysum += res.results[c]["y"].astype(np.float32)

    out = np.zeros((N, D), np.float32)
    for s, e in enumerate(order):
        L = int(cnts[e])
        ye = np.empty((L, D), np.float32)
        for g, (sg, t0, tsz) in enumerate(gl):
            if sg != s:
                continue
            blk = ysum[g * P:(g + 1) * P, :ND * tsz]
            ye[t0:t0 + tsz] = (
                blk.reshape(P, ND, tsz).transpose(2, 1, 0).reshape(tsz, D))
        yv = ye + b2[e][None, :].astype(np.float32)
        out[idx_e[e]] += wts_e[e][:, None] * yv
    return out.reshape(B, T, D_)
